# revision 69
# baseline (speedup 1.0000x reference)
"""Trainium2 Bass kernel for nn_PointSetAnchorPoseHead (NMS pose decode).

Runs on 8 NeuronCores via run_bass_kernel_spmd. See bottom for host glue.

Algorithm (per core, SPMD):
  heat stage: rows sharded 64/core (+2 halo). 5x5 maxpool via shifted-max
  cascades on (col,row)-in-free layout, work split across DVE and Pool
  engines by column ranges; exact key packing: for values v>t (t=1-2^-11)
  key = (v-t)*2^35 + (2047-slabidx), a 24-bit exact f32 int. max8 per
  (joint, 25-col slab) -> per-core top-16. The core then decodes ALL its
  slab candidates, eq-match-selects gpos for its top-16, gathers the
  matching offsets from its offset shard (one indirect DMA), and packs
  (key | gpos | offx | offy) into a single AllGather. After the gather
  every core replicates the merge: top-30 keys via max8+match_replace,
  then a partition-spread eq-match select pulls (gpos, offx, offy) for
  the 30 winners. Only ONE collective total (no AllReduce).
  pose stage: 98 tiles of 128 poses, software-pipelined 4 deep:
  score = |c|^2 - 2 q.c in one fp32 PE matmul (block-diag W built on
  device; pose transposes precomputed during the heat phase on idle
  PE/Act). rmin on DVE, one-hot is_lt on Pool, 4 bf16 PE transposes into
  one PSUM tile, single Act copy, 3-way bf16-split gather matmul, Pool
  copy to a 4-tile staging buffer, batched output DMA. Host recomputes
  the few count!=1 sites exactly (reference f32 arithmetic).
"""

import numpy as np

J = 17
K = 30
H = 512
W = 512
NCORES = 8
RPC = H // NCORES          # 64 rows per core
PT = 128
NT = 98
NPAD = PT * NT             # 12544
CAUG = 36                  # x17, y17, 1, 0
JK = J * K                 # 510
JKP = 512
SW = 25                    # slab width; 21 slabs
NSLAB = 21
SLABW = SW + 4             # stored cols (2 halo each side, 29)
RW = RPC + 4               # stored rows (68)
SLABF = SW * RPC           # 1600 owned cells
SPT = 7                    # slabs per heat partition-tile
NTILE_H = 3
THRESH_T = float(1.0 - 2.0 ** -11)
KEYSCALE = float(2.0 ** 35)
SCORE_THRESH = 32.0
NEG = -1.0e30
OFFSZ = J * 2 * RPC * W

_CACHE = {}
LAST_EXEC_NS = None


# --------------------------------------------------------------------------
# device program
# --------------------------------------------------------------------------
def _build_program(stride, ntiles=NT, debug=False):
    import concourse.bass as bass
    import concourse.bacc as bacc
    import concourse.mybir as mybir
    from concourse import tile

    dt = mybir.dt
    Alu = mybir.AluOpType
    Ax = mybir.AxisListType
    nc = bacc.Bacc(None)

    def din(name, shape, dtype=dt.float32):
        return nc.declare_dram_parameter(name, list(shape), dtype, isOutput=False)

    poses_d = din("poses", [NPAD, CAUG])
    heat_d = din("heat", [NTILE_H * PT, SLABW * RW])
    offs_d = din("offs", [OFFSZ, 1])
    cconst_d = din("coreconst", [J, 2])
    identf_d = din("identf", [PT, PT])
    identb_d = din("identb", [PT, PT], dt.bfloat16)
    rev_d = din("revconst", [PT, SLABF])
    mtx_d = din("maskTx", [PT, 4 * 51])
    mty_d = din("maskTy", [PT, 4 * 51])
    mtc_d = din("maskTc", [PT, 4 * 51])
    cgidx_d = din("cgidx", [J, NSLAB * 8])
    wz_d = din("wzero", [CAUG, JKP])
    cz_d = din("czero", [2, JKP])

    out_d = nc.declare_dram_parameter("out", [NPAD, 51], dt.float32, isOutput=True)
    cand_d = nc.declare_dram_parameter("cand", [J, PT], dt.float32, isOutput=True)
    if debug:
        dbg_d = nc.declare_dram_parameter("dbg", [J, 1024], dt.float32,
                                          isOutput=True)
        dbg2_d = nc.declare_dram_parameter("dbg2", [102, 64], dt.float32,
                                           isOutput=True)

    with tile.TileContext(nc) as tc:
        with (
            tc.tile_pool(name="const", bufs=1) as cpool,
            tc.tile_pool(name="heatp", bufs=2) as hpool,
            tc.tile_pool(name="work", bufs=1) as wpool,
            tc.tile_pool(name="small", bufs=1) as spool,
            tc.tile_pool(name="pose", bufs=1) as ppool,
            tc.tile_pool(name="loop", bufs=2) as lpool,
            tc.tile_pool(name="merge", bufs=1) as mpool,
            tc.tile_pool(name="psA", bufs=2, space="PSUM") as psA,
            tc.tile_pool(name="psB", bufs=2, space="PSUM") as psB,
            tc.tile_pool(name="dram", bufs=1, space="DRAM") as dpool,
        ):
            # ---------- heat tile DMAs first (don't sit behind poses DMA) ----
            hx_tiles = []
            for ti in range(2):
                hx = hpool.tile([PT, SLABW * RW], dt.float32, tag="heat")
                if ti == 0:
                    nc.sync.dma_start(hx[:, 0:15 * RW],
                                      heat_d[0:PT, 0:15 * RW])
                    nc.sync.dma_start(hx[:, 15 * RW:],
                                      heat_d[0:PT, 15 * RW:])
                else:
                    nc.sync.dma_start(hx[:], heat_d[ti * PT:(ti + 1) * PT, :])
                hx_tiles.append(hx)

            # ---------- constants ----------
            identf = cpool.tile([PT, PT], dt.float32)
            nc.sync.dma_start(identf[:], identf_d[:])
            identb = cpool.tile([PT, PT], dt.bfloat16)
            nc.sync.dma_start(identb[:], identb_d[:])
            rev = cpool.tile([PT, SLABF], dt.float32)
            nc.sync.dma_start(rev[:], rev_d[:])
            mtx = cpool.tile([PT, 4 * 51], dt.float32)
            nc.sync.dma_start(mtx[:], mtx_d[:])
            mty = cpool.tile([PT, 4 * 51], dt.float32)
            nc.sync.dma_start(mty[:], mty_d[:])
            mtc = cpool.tile([PT, 4 * 51], dt.float32)
            nc.sync.dma_start(mtc[:], mtc_d[:])
            cconst = cpool.tile([J, 2], dt.float32)
            nc.sync.dma_start(cconst[:], cconst_d[:])
            cg_f = cpool.tile([J, NSLAB * 8], dt.float32)
            nc.sync.dma_start(cg_f[:], cgidx_d[:])
            # spread constants used later (ready immediately)
            r0b = cpool.tile([J * 6, 1], dt.float32)
            nc.sync.dma_start(
                bass.AP(r0b.tensor, 0, [[1, J * 6], [1, 1]]),
                bass.AP(cconst.tensor, 1, [[2, J], [0, 6], [1, 1]]))
            jr_i = cpool.tile([J, 1], dt.int32)
            nc.gpsimd.iota(jr_i[:], pattern=[[0, 1]], base=0, channel_multiplier=1)
            jrowf = cpool.tile([J, 1], dt.float32)
            nc.vector.tensor_copy(jrowf[:], jr_i[:])
            j2 = cpool.tile([J, 1], dt.float32)
            nc.vector.tensor_scalar_mul(j2[:], jrowf[:], 2.0)
            j2b = cpool.tile([J * 6, 1], dt.float32)
            nc.sync.dma_start(
                bass.AP(j2b.tensor, 0, [[1, J * 6], [1, 1]]),
                bass.AP(j2.tensor, 0, [[1, J], [0, 6], [1, 1]]))

            posesb = ppool.tile([PT, NT * CAUG], dt.float32)
            nc.sync.dma_start(
                posesb[:],
                bass.AP(poses_d[:].tensor, 0,
                        [[CAUG, PT], [PT * CAUG, NT], [1, CAUG]]))

            # ---------- pose transposes precomputed on idle PE/Act ----------
            NB2 = (ntiles + 1) // 2
            posesT = ppool.tile([64 + CAUG, NB2 * PT], dt.float32)

            def posesT_slice(t):
                ch, blk = t % 2, t // 2
                return posesT[ch * 64:ch * 64 + CAUG,
                              blk * PT:(blk + 1) * PT]

            for t in range(ntiles):
                pT_ps = psB.tile([CAUG, PT], dt.float32, tag="psb", bufs=2)
                nc.tensor.transpose(pT_ps[:], posesb[:, t * CAUG:(t + 1) * CAUG],
                                    identf[:])
                nc.scalar.copy(posesT_slice(t), pT_ps[:])

            # ---------- heat stage (DVE/Pool split by slab columns) ----------
            def ap(t, coff, roff, ccnt, rcnt, rw):
                return bass.AP(t.tensor, coff * rw + roff,
                               [[t.shape[1], PT], [rw, ccnt], [1, rcnt]])

            def split_tt(op, out_t, rw_o, in0_t, co0, ro0, rw0,
                         in1_t, co1, ro1, rw1, ncols, rcnt, dcols):
                nc.vector.tensor_tensor(
                    out=ap(out_t, 0, 0, ncols, rcnt, rw_o),
                    in0=ap(in0_t, co0, ro0, ncols, rcnt, rw0),
                    in1=ap(in1_t, co1, ro1, ncols, rcnt, rw1), op=op)

            kall_ps = psA.tile([J, NSLAB * 8], dt.float32, tag="gps", bufs=2)
            kall = spool.tile([J, NSLAB * 8], dt.float32)
            for ti in range(NTILE_H):
                if ti < 2:
                    hx = hx_tiles[ti]
                else:
                    hx = hpool.tile([PT, SLABW * RW], dt.float32, tag="heat")
                    nc.sync.dma_start(hx[:], heat_d[ti * PT:(ti + 1) * PT, :])

                m1 = wpool.tile([PT, SLABW * 67], dt.float32, tag="m1")
                if ti == 0:
                    for c0, cn in ((0, 15), (15, SLABW - 15)):
                        nc.vector.tensor_tensor(
                            out=ap(m1, c0, 0, cn, 67, 67),
                            in0=ap(hx, c0, 0, cn, 67, RW),
                            in1=ap(hx, c0, 1, cn, 67, RW), op=Alu.max)
                else:
                    split_tt(Alu.max, m1, 67, hx, 0, 0, RW, hx, 0, 1, RW,
                             SLABW, 67, 17)
                m2 = wpool.tile([PT, SLABW * 65], dt.float32, tag="m2")
                split_tt(Alu.max, m2, 65, m1, 0, 0, 67, m1, 0, 2, 67,
                         SLABW, 65, 17)
                w5r = wpool.tile([PT, SLABW * RPC], dt.float32, tag="w5r")
                split_tt(Alu.max, w5r, RPC, m2, 0, 0, 65, hx, 0, 4, RW,
                         SLABW, RPC, 17)
                n1 = wpool.tile([PT, 28 * RPC], dt.float32, tag="n1")
                split_tt(Alu.max, n1, RPC, w5r, 0, 0, RPC, w5r, 1, 0, RPC,
                         28, RPC, 16)
                n2 = wpool.tile([PT, 26 * RPC], dt.float32, tag="n2")
                split_tt(Alu.max, n2, RPC, n1, 0, 0, RPC, n1, 2, 0, RPC,
                         26, RPC, 15)
                w55 = wpool.tile([PT, SW * RPC], dt.float32, tag="w55")
                split_tt(Alu.max, w55, RPC, n2, 0, 0, RPC, w5r, 4, 0, RPC,
                         SW, RPC, 13)
                eq = wpool.tile([PT, SW * RPC], dt.float32, tag="n1")
                split_tt(Alu.is_equal, eq, RPC, hx, 2, 2, RW, w55, 0, 0, RPC,
                         SW, RPC, 13)
                r1 = wpool.tile([PT, SW * RPC], dt.float32, tag="m2")
                keyt = wpool.tile([PT, SLABF], dt.float32, tag="m1")
                nc.vector.scalar_tensor_tensor(
                    out=ap(r1, 0, 0, SW, RPC, RPC),
                    in0=ap(hx, 2, 2, SW, RPC, RW), scalar=-THRESH_T,
                    in1=ap(eq, 0, 0, SW, RPC, RPC),
                    op0=Alu.add, op1=Alu.mult)
                nc.vector.scalar_tensor_tensor(
                    out=keyt[:], in0=r1[:], scalar=KEYSCALE, in1=rev[:],
                    op0=Alu.mult, op1=Alu.add)
                k8t = wpool.tile([PT, 8], dt.float32, tag="k8t")
                nc.vector.max(k8t[:], keyt[:])
                # regroup k8t [cgl*17+j, v] -> kall[j, (ti*7+cgl)*8+v] with
                # one-hot selector matmuls on the idle PE (exact for 0/1 wts)
                for cgl in range(SPT):
                    nc.tensor.matmul(
                        kall_ps[:, (ti * SPT + cgl) * 8:(ti * SPT + cgl + 1) * 8],
                        identf[0:SPT * J, cgl * J:(cgl + 1) * J],
                        k8t[0:SPT * J, :], start=True, stop=True)

            # ---------- per-core top-16 ----------
            nc.scalar.copy(kall[:], kall_ps[:])
            kwork = spool.tile([J, NSLAB * 8], dt.float32)
            nc.vector.tensor_copy(kwork[:], kall[:])
            key16p = spool.tile([J, 18], dt.float32)
            nc.vector.memset(key16p[:], NEG)
            key16 = key16p[:, 0:16]
            nc.vector.max(key16p[:, 0:8], kwork[:])
            nc.vector.match_replace(kwork[:], key16p[:, 0:8], kwork[:], NEG)
            nc.vector.max(key16p[:, 8:16], kwork[:])
            # keys-only AllGather fires immediately; it overlaps the rest of
            # the pre-collective work (decode/select/offset gather)
            ag1_in = dpool.tile([J, 18], dt.float32)
            ag1_out = dpool.tile([NCORES * J, 18], dt.float32)
            nc.scalar.dma_start(ag1_in[:], key16p[:])
            nc.gpsimd.collective_compute(
                "AllGather", Alu.bypass,
                replica_groups=[list(range(NCORES))],
                ins=[ag1_in[:]], outs=[ag1_out[:]])

            # decode all local per-slab candidates -> gposall [17, 168]
            ki = spool.tile([J, NSLAB * 8], dt.int32)
            kclamp = spool.tile([J, NSLAB * 8], dt.float32)
            nc.vector.tensor_scalar_max(kclamp[:], kall[:], 0.0)
            nc.vector.tensor_copy(ki[:], kclamp[:])
            s11 = spool.tile([J, NSLAB * 8], dt.int32)
            nc.vector.tensor_scalar(out=s11[:], in0=ki[:], scalar1=2047,
                                    scalar2=None, op0=Alu.bitwise_and)
            nc.vector.tensor_scalar(out=s11[:], in0=s11[:], scalar1=-2047,
                                    scalar2=-1, op0=Alu.add, op1=Alu.mult)
            ci = spool.tile([J, NSLAB * 8], dt.int32)
            nc.vector.tensor_scalar(out=ci[:], in0=s11[:], scalar1=6,
                                    scalar2=None, op0=Alu.arith_shift_right)
            ri = spool.tile([J, NSLAB * 8], dt.int32)
            nc.vector.tensor_scalar(out=ri[:], in0=s11[:], scalar1=RPC - 1,
                                    scalar2=None, op0=Alu.bitwise_and)
            cf = spool.tile([J, NSLAB * 8], dt.float32)
            nc.vector.tensor_copy(cf[:], ci[:])
            rf = spool.tile([J, NSLAB * 8], dt.float32)
            nc.vector.tensor_copy(rf[:], ri[:])
            gcol = spool.tile([J, NSLAB * 8], dt.float32)
            nc.vector.scalar_tensor_tensor(out=gcol[:], in0=cg_f[:], scalar=float(SW),
                                           in1=cf[:], op0=Alu.mult, op1=Alu.add)
            gposall = spool.tile([J, NSLAB * 8], dt.float32)
            nc.vector.scalar_tensor_tensor(out=gposall[:], in0=rf[:], scalar=float(W),
                                           in1=gcol[:], op0=Alu.mult, op1=Alu.add)
            nc.vector.tensor_scalar(out=gposall[:], in0=gposall[:],
                                    scalar1=cconst[:, 0:1], scalar2=None, op0=Alu.add)

            # ---------- select gpos for top-16 (partition-spread match) -----
            # p = j*6+kk handles keys 3kk..3kk+2; each vs all 168 candidates
            NSP = NSLAB * 8            # 168
            kall_sp = spool.tile([J * 6, NSP], dt.float32)
            nc.sync.dma_start(
                kall_sp[:],
                bass.AP(kall.tensor, 0, [[NSP, J], [0, 6], [1, NSP]]))
            k16_sp = spool.tile([J * 6, 3], dt.float32)
            nc.sync.dma_start(
                bass.AP(k16_sp.tensor, 0, [[3, J * 6], [1, 3]]),
                bass.AP(key16p.tensor, 0, [[18, J], [1, 18]]))
            gpos_sp = spool.tile([J * 6, NSP], dt.float32)
            nc.sync.dma_start(
                gpos_sp[:],
                bass.AP(gposall.tensor, 0, [[NSP, J], [0, 6], [1, NSP]]))
            eq16 = spool.tile([J * 6, 3 * NSP], dt.float32)
            nc.vector.tensor_tensor(
                out=bass.AP(eq16.tensor, 0, [[3 * NSP, J * 6], [NSP, 3], [1, NSP]]),
                in0=bass.AP(k16_sp.tensor, 0, [[3, J * 6], [1, 3], [0, NSP]]),
                in1=bass.AP(kall_sp.tensor, 0, [[NSP, J * 6], [0, 3], [1, NSP]]),
                op=Alu.is_equal)
            nc.vector.tensor_tensor(
                out=eq16[:], in0=eq16[:],
                in1=bass.AP(gpos_sp.tensor, 0, [[NSP, J * 6], [0, 3], [1, NSP]]),
                op=Alu.mult)
            g16_sp = spool.tile([J * 6, 3], dt.float32)
            nc.vector.tensor_reduce(
                g16_sp[:],
                bass.AP(eq16.tensor, 0, [[3 * NSP, J * 6], [NSP, 3], [1, NSP]]),
                axis=Ax.X, op=Alu.add)
            # valid/decode/idx all in spread layout [102, 3]
            valid_sp = spool.tile([J * 6, 3], dt.float32)
            nc.vector.tensor_scalar(out=valid_sp[:], in0=g16_sp[:], scalar1=0.5,
                                    scalar2=None, op0=Alu.is_gt)
            g16m_sp = spool.tile([J * 6, 3], dt.float32)
            nc.vector.tensor_tensor(out=g16m_sp[:], in0=g16_sp[:], in1=valid_sp[:],
                                    op=Alu.mult)
            gq = spool.tile([J * 6, 3], dt.int32)
            nc.vector.tensor_copy(gq[:], g16m_sp[:])
            gqr = spool.tile([J * 6, 3], dt.int32)
            nc.vector.tensor_scalar(out=gqr[:], in0=gq[:], scalar1=9,
                                    scalar2=None, op0=Alu.arith_shift_right)
            rowf = spool.tile([J * 6, 3], dt.float32)
            nc.vector.tensor_copy(rowf[:], gqr[:])
            gqc = spool.tile([J * 6, 3], dt.int32)
            nc.vector.tensor_scalar(out=gqc[:], in0=gq[:], scalar1=W - 1,
                                    scalar2=None, op0=Alu.bitwise_and)
            colf = spool.tile([J * 6, 3], dt.float32)
            nc.vector.tensor_copy(colf[:], gqc[:])
            lrow = spool.tile([J * 6, 3], dt.float32)
            nc.vector.tensor_scalar(out=lrow[:], in0=rowf[:],
                                    scalar1=r0b[:, 0:1], scalar2=None,
                                    op0=Alu.subtract)
            nc.vector.tensor_scalar(out=lrow[:], in0=lrow[:], scalar1=0.0,
                                    scalar2=float(RPC - 1), op0=Alu.max, op1=Alu.min)
            # offs shard is transposed on host to [lrow, col, j, 2]:
            # site addr = (lrow*W + col)*2J + 2j; gathers (offx, offy) adjacent
            idx_sp = spool.tile([J * 6, 3], dt.float32)
            nc.vector.scalar_tensor_tensor(out=idx_sp[:], in0=lrow[:],
                                           scalar=float(W), in1=colf[:],
                                           op0=Alu.mult, op1=Alu.add)
            nc.vector.scalar_tensor_tensor(out=idx_sp[:], in0=idx_sp[:],
                                           scalar=float(2 * J),
                                           in1=bass.AP(j2b.tensor, 0,
                                                       [[1, J * 6], [1, 1], [0, 3]]),
                                           op0=Alu.mult, op1=Alu.add)
            idxp = spool.tile([J * 6, 3], dt.int32)
            nc.vector.tensor_copy(idxp[:], idx_sp[:])
            offs_g = spool.tile([J * 6, 6], dt.float32)
            nc.vector.memset(offs_g[:], 0.0)
            for c in range(3):
                nc.gpsimd.indirect_dma_start(
                    out=offs_g[:, 2 * c:2 * c + 2], out_offset=None,
                    in_=offs_d[:],
                    in_offset=bass.IndirectOffsetOnAxis(ap=idxp[:, c:c + 1],
                                                        axis=0),
                    bounds_check=OFFSZ - 2, oob_is_err=False)
            ogm = spool.tile([J * 6, 6], dt.float32)
            nc.vector.tensor_tensor(
                out=bass.AP(ogm.tensor, 0, [[6, J * 6], [2, 3], [1, 2]]),
                in0=bass.AP(offs_g.tensor, 0, [[6, J * 6], [2, 3], [1, 2]]),
                in1=bass.AP(valid_sp.tensor, 0, [[3, J * 6], [1, 3], [0, 2]]),
                op=Alu.mult)

            # ---------- AllGather 2: gpos18 | offx18 | offy18 ----------
            ag_in = dpool.tile([J, 54], dt.float32)
            ag_out = dpool.tile([NCORES * J, 54], dt.float32)
            nc.scalar.dma_start(
                bass.AP(ag_in[:].tensor, 0, [[54, J], [1, 18]]),
                bass.AP(g16m_sp.tensor, 0, [[3, J * 6], [1, 3]]))
            nc.scalar.dma_start(
                bass.AP(ag_in[:].tensor, 18, [[54, J], [1, 18]]),
                bass.AP(ogm.tensor, 0, [[6, J * 6], [2, 3]]))
            nc.sync.dma_start(
                bass.AP(ag_in[:].tensor, 36, [[54, J], [1, 18]]),
                bass.AP(ogm.tensor, 1, [[6, J * 6], [2, 3]]))
            nc.gpsimd.collective_compute(
                "AllGather", Alu.bypass,
                replica_groups=[list(range(NCORES))],
                ins=[ag_in[:]], outs=[ag_out[:]])

            NCF = NCORES * 18          # 144
            NC16 = NCF
            # key pool from AG1 (available during AG2); merge + eqf overlap AG2
            kpool_t = mpool.tile([J, NCF], dt.float32, name="pt_kpool")
            nc.sync.dma_start(
                kpool_t[:],
                bass.AP(ag1_out.tensor, 0,
                        [[18, J], [J * 18, NCORES], [1, 18]]))
            pools = {"kpool": kpool_t}
            sp = {}
            sp_k = mpool.tile([J * 6, NC16], dt.float32, name="sp_kpool")
            nc.sync.dma_start(
                bass.AP(sp_k.tensor, 0, [[NC16, J * 6], [1, NC16]]),
                bass.AP(kpool_t.tensor, 0, [[NC16, J], [0, 6], [1, NC16]]))
            sp["kpool"] = sp_k

            # ---------- merge: top-32 keys (use first 30) ----------
            kmw = mpool.tile([J, NCF], dt.float32)
            nc.vector.tensor_copy(kmw[:], pools["kpool"][:])
            fkeys = mpool.tile([J, 32], dt.float32)
            for r in range(4):
                nc.vector.max(fkeys[:, r * 8:(r + 1) * 8], kmw[:])
                if r < 3:
                    nc.vector.match_replace(kmw[:], fkeys[:, r * 8:(r + 1) * 8],
                                            kmw[:], NEG)

            # payload pools from AG2
            for fi, nm in enumerate(("gpool", "oxpool", "oypool")):
                pt = mpool.tile([J, NCF], dt.float32, name=f"pt_{nm}")
                nc.sync.dma_start(
                    pt[:],
                    bass.AP(ag_out.tensor, fi * 18,
                            [[54, J], [J * 54, NCORES], [1, 18]]))
                pools[nm] = pt
            for nm in ("gpool", "oxpool", "oypool"):
                t = mpool.tile([J * 6, NC16], dt.float32, name=f"sp_{nm}")
                nc.sync.dma_start(
                    bass.AP(t.tensor, 0, [[NC16, J * 6], [1, NC16]]),
                    bass.AP(pools[nm].tensor, 0, [[NC16, J], [0, 6], [1, NC16]]))
                sp[nm] = t

            # ---------- spread select of (gpos, offx, offy) for 30 winners --
            # p = j*6+kk handles final keys kk*5..kk*5+4, each vs 144 cands
            fk_sp = mpool.tile([J * 6, 5], dt.float32)
            nc.scalar.dma_start(
                bass.AP(fk_sp.tensor, 0, [[5, J * 6], [1, 5]]),
                bass.AP(fkeys.tensor, 0, [[32, J], [1, 30]]))
            eqf = mpool.tile([J * 6, 5 * NC16], dt.float32)
            nc.vector.tensor_tensor(
                out=bass.AP(eqf.tensor, 0, [[5 * NC16, J * 6], [NC16, 5], [1, NC16]]),
                in0=bass.AP(fk_sp.tensor, 0, [[5, J * 6], [1, 5], [0, NC16]]),
                in1=bass.AP(sp["kpool"].tensor, 0,
                            [[NC16, J * 6], [0, 5], [1, NC16]]),
                op=Alu.is_equal)
            sel3 = mpool.tile([J * 6, 15], dt.float32)
            prods = [mpool.tile([J * 6, 5 * NC16], dt.float32, name="prodA"),
                     mpool.tile([J * 6, 5 * NC16], dt.float32, name="prodB")]
            for pi, (nm, eng) in enumerate((("gpool", nc.vector),
                                            ("oxpool", nc.vector),
                                            ("oypool", nc.vector))):
                prod = prods[pi % 2]
                eng.tensor_tensor(
                    out=prod[:], in0=eqf[:],
                    in1=bass.AP(sp[nm].tensor, 0,
                                [[NC16, J * 6], [0, 5], [1, NC16]]),
                    op=Alu.mult)
                nc.vector.tensor_reduce(
                    sel3[:, pi * 5:(pi + 1) * 5],
                    bass.AP(prod.tensor, 0, [[5 * NC16, J * 6], [NC16, 5], [1, NC16]]),
                    axis=Ax.X, op=Alu.add)
            # decode directly in spread layout [102, 5] (p = j*6+kk)

            # final candidate coords (reference arithmetic: stride*(x+off))
            yq = spool.tile([J * 6, 5], dt.int32)
            nc.vector.tensor_copy(yq[:], sel3[:, 0:5])
            yqs = spool.tile([J * 6, 5], dt.int32)
            nc.vector.tensor_scalar(out=yqs[:], in0=yq[:], scalar1=9,
                                    scalar2=None, op0=Alu.arith_shift_right)
            yf = spool.tile([J * 6, 5], dt.float32)
            nc.vector.tensor_copy(yf[:], yqs[:])
            xq = spool.tile([J * 6, 5], dt.int32)
            nc.vector.tensor_scalar(out=xq[:], in0=yq[:], scalar1=W - 1,
                                    scalar2=None, op0=Alu.bitwise_and)
            xf = spool.tile([J * 6, 5], dt.float32)
            nc.vector.tensor_copy(xf[:], xq[:])
            hxc = spool.tile([J * 6, 5], dt.float32)
            hyc = spool.tile([J * 6, 5], dt.float32)
            nc.vector.tensor_tensor(out=hxc[:], in0=xf[:], in1=sel3[:, 5:10],
                                    op=Alu.add)
            nc.vector.tensor_scalar_mul(hxc[:], hxc[:], float(stride))
            nc.vector.tensor_tensor(out=hyc[:], in0=yf[:], in1=sel3[:, 10:15],
                                    op=Alu.add)
            nc.vector.tensor_scalar_mul(hyc[:], hyc[:], float(stride))
            # cand output: DRAM-side reorder [j*6+kk, q] -> cand[j, kk*5+q]
            for sap, c0 in ((hxc[:], 0), (hyc[:], K), (sel3[:, 0:5], 2 * K)):
                nc.gpsimd.dma_start(
                    bass.AP(cand_d[:].tensor, c0, [[PT, J], [5, 6], [1, 5]]),
                    sap)

            if debug:
                nc.sync.dma_start(dbg_d[:, 0:168], kall[:])
                nc.sync.dma_start(dbg_d[:, 168:336], gposall[:])
                nc.sync.dma_start(dbg_d[:, 336:354], key16p[:])
                nc.sync.dma_start(
                    bass.AP(dbg_d[:].tensor, 354, [[1024, J], [1, 18]]),
                    bass.AP(g16m_sp.tensor, 0, [[3, J * 6], [1, 3]]))
                nc.sync.dma_start(dbg_d[:, 388:532], pools["kpool"][:])
                nc.sync.dma_start(dbg_d[:, 532:676], pools["gpool"][:])
                nc.sync.dma_start(dbg_d[:, 676:820], pools["oxpool"][:])
                nc.sync.dma_start(dbg_d[:, 836:868], fkeys[:])
                nc.sync.dma_start(dbg2_d[:, 0:15], sel3[:])
                nc.sync.dma_start(dbg2_d[:, 15:20], fk_sp[:])
                nc.sync.dma_start(dbg2_d[:, 20:25], hxc[:])

            # ---------- score matrix W [36, 512] assembled in DRAM ----------
            # wz_d is host-zeroed; write the block-diagonal parts:
            #   row j cols j*30+k = -2cx; row 17+j = -2cy; row 34 = cx^2+cy^2
            m2x = spool.tile([J * 6, 5], dt.float32)
            nc.vector.tensor_scalar_mul(m2x[:], hxc[:], -2.0)
            m2y = spool.tile([J * 6, 5], dt.float32)
            nc.vector.tensor_scalar_mul(m2y[:], hyc[:], -2.0)
            cx2 = spool.tile([J * 6, 5], dt.float32)
            nc.vector.tensor_tensor(out=cx2[:], in0=hxc[:], in1=hxc[:], op=Alu.mult)
            c2s = spool.tile([J * 6, 5], dt.float32)
            nc.vector.scalar_tensor_tensor(out=c2s[:], in0=hyc[:], scalar=1.0,
                                           in1=hyc[:], op0=Alu.mult, op1=Alu.mult)
            nc.vector.tensor_tensor(out=c2s[:], in0=c2s[:], in1=cx2[:], op=Alu.add)
            nc.sync.dma_start(
                bass.AP(wz_d[:].tensor, 0, [[JKP + K, J], [5, 6], [1, 5]]),
                m2x[:])
            nc.scalar.dma_start(
                bass.AP(wz_d[:].tensor, J * JKP, [[JKP + K, J], [5, 6], [1, 5]]),
                m2y[:])
            nc.sync.dma_start(
                bass.AP(wz_d[:].tensor, 34 * JKP, [[K, J], [5, 6], [1, 5]]),
                c2s[:])
            wmat = spool.tile([64 + CAUG, JKP], dt.float32)
            nc.scalar.dma_start(wmat[0:CAUG, :], wz_d[:])
            nc.sync.dma_start(wmat[64:64 + CAUG, :], wz_d[:])
            # candidate coords chunked [128, 4] via host-zeroed DRAM bounce
            nc.gpsimd.dma_start(
                bass.AP(cz_d[:].tensor, 0, [[K, J], [5, 6], [1, 5]]), hxc[:])
            nc.gpsimd.dma_start(
                bass.AP(cz_d[:].tensor, JKP, [[K, J], [5, 6], [1, 5]]), hyc[:])

            # ---------- gather table T (bf16 3-split) ----------
            cxP = spool.tile([PT, 4], dt.float32)
            cyP = spool.tile([PT, 4], dt.float32)
            nc.sync.dma_start(
                cxP[:], bass.AP(cz_d[:].tensor, 0, [[1, PT], [PT, 4]]))
            nc.sync.dma_start(
                cyP[:], bass.AP(cz_d[:].tensor, JKP, [[1, PT], [PT, 4]]))
            t_y = spool.tile([PT, 4 * 51], dt.float32)
            tfull = spool.tile([PT, 4 * 51], dt.float32)
            for c in range(4):
                nc.vector.scalar_tensor_tensor(
                    out=t_y[:, c * 51:(c + 1) * 51],
                    in0=mty[:, c * 51:(c + 1) * 51], scalar=cyP[:, c:c + 1],
                    in1=mtc[:, c * 51:(c + 1) * 51], op0=Alu.mult, op1=Alu.add)
                nc.vector.scalar_tensor_tensor(
                    out=tfull[:, c * 51:(c + 1) * 51],
                    in0=mtx[:, c * 51:(c + 1) * 51], scalar=cxP[:, c:c + 1],
                    in1=t_y[:, c * 51:(c + 1) * 51], op0=Alu.mult, op1=Alu.add)
            t_hi = spool.tile([PT, 4 * 51], dt.bfloat16)
            t_mid = spool.tile([PT, 4 * 51], dt.bfloat16)
            t_lo = spool.tile([PT, 4 * 51], dt.bfloat16)
            tr1 = spool.tile([PT, 4 * 51], dt.float32)
            tr1b = spool.tile([PT, 4 * 51], dt.float32)
            nc.vector.tensor_copy(t_hi[:], tfull[:])
            nc.vector.tensor_copy(tr1b[:], t_hi[:])
            nc.vector.tensor_tensor(out=tr1[:], in0=tfull[:], in1=tr1b[:],
                                    op=Alu.subtract)
            nc.vector.tensor_copy(t_mid[:], tr1[:])
            nc.vector.tensor_copy(tr1b[:], t_mid[:])
            nc.vector.tensor_tensor(out=tr1[:], in0=tr1[:], in1=tr1b[:],
                                    op=Alu.subtract)
            nc.vector.tensor_copy(t_lo[:], tr1[:])

            # ---------- pose loop (software-pipelined, 5 stages) ----------
            thrc = spool.tile([PT, 1], dt.float32)
            nc.vector.memset(thrc[:], SCORE_THRESH)
            score_t = {}
            oh_t = {}
            ohT_t = {}
            gst_ref = [None]

            def st_score(t):
                s = psA.tile([PT, JKP], dt.float32, tag="score", bufs=3)
                ch = t % 2
                nc.tensor.matmul(s[:], posesT_slice(t),
                                 wmat[ch * 64:ch * 64 + CAUG, :],
                                 start=True, stop=True)
                score_t[t] = s

            rminp_t = {}

            def st_rmin(t):
                s = score_t[t]
                sc3 = bass.AP(s.tensor, 0, [[JKP, PT], [K, J], [1, K]])
                rmin = lpool.tile([PT, J], dt.float32, tag="rmin")
                nc.vector.tensor_reduce(rmin[:], sc3, axis=Ax.X, op=Alu.min)
                rminp = lpool.tile([PT, J], dt.float32, tag="rminp")
                nc.scalar.add(rminp[:], rmin[:], thrc[:, 0:1])
                rminp_t[t] = rminp

            def st_islt(t):
                s = score_t.pop(t)
                rminp = rminp_t.pop(t)
                sc3 = bass.AP(s.tensor, 0, [[JKP, PT], [K, J], [1, K]])
                oh = lpool.tile([PT, JKP], dt.bfloat16, tag="oh")
                nc.scalar.memzero(oh[:, JK:JKP])
                rb = bass.AP(rminp.tensor, 0, [[J, PT], [1, J], [0, K]])
                nc.vector.tensor_tensor(
                    out=bass.AP(oh.tensor, 0, [[JKP, PT], [K, J], [1, K]]),
                    in0=sc3, in1=rb, op=Alu.is_lt)
                oh_t[t] = oh

            def st_transp(t):
                oh = oh_t.pop(t)
                ohT_ps = psA.tile([PT, JKP], dt.bfloat16, tag="ohTps", bufs=1)
                for c in range(4):
                    nc.tensor.transpose(ohT_ps[:, c * PT:(c + 1) * PT],
                                        oh[:, c * PT:(c + 1) * PT], identb[:])
                ohT = lpool.tile([PT, JKP], dt.bfloat16, tag="ohT")
                nc.scalar.copy(ohT[:], ohT_ps[:])
                ohT_t[t] = ohT

            def st_gather(t):
                ohT = ohT_t.pop(t)
                g_ps = psA.tile([PT, 51], dt.float32, tag="gps")
                for c in range(4):
                    for si, s in enumerate((t_hi, t_mid, t_lo)):
                        nc.tensor.matmul(g_ps[:], ohT[:, c * PT:(c + 1) * PT],
                                         s[:, c * 51:(c + 1) * 51],
                                         start=(c == 0 and si == 0),
                                         stop=(c == 3 and si == 2))
                slot = t % 4
                if slot == 0:
                    gst_ref[0] = lpool.tile([PT, 4 * 51], dt.float32, tag="gst",
                                            name="gst")
                gst = gst_ref[0]
                nc.scalar.copy(gst[:, slot * 51:(slot + 1) * 51], g_ps[:])
                if slot == 3 or t == ntiles - 1:
                    nb = slot + 1
                    t0 = t - slot
                    nc.sync.dma_start(
                        bass.AP(out_d[:].tensor, t0 * PT * 51,
                                [[51, PT], [PT * 51, nb], [1, 51]]),
                        bass.AP(gst.tensor, 0, [[4 * 51, PT], [51, nb], [1, 51]]))

            for t in range(ntiles + 4):
                if t < ntiles:
                    st_score(t)
                if 1 <= t < ntiles + 1:
                    st_rmin(t - 1)
                if 2 <= t < ntiles + 2:
                    st_islt(t - 2)
                if 3 <= t < ntiles + 3:
                    st_transp(t - 3)
                if t >= 4:
                    st_gather(t - 4)

    nc.compile()
    return nc


# --------------------------------------------------------------------------
# host-side constants / shards
# --------------------------------------------------------------------------
def _build_consts():
    import ml_dtypes
    c = {}
    c["identf"] = np.eye(PT, dtype=np.float32)
    c["identb"] = np.eye(PT, dtype=np.float32).astype(ml_dtypes.bfloat16)
    s = (np.arange(SW)[:, None] * RPC + np.arange(RPC)[None, :]).reshape(-1)
    c["revconst"] = np.broadcast_to((2047 - s).astype(np.float32),
                                    (PT, SLABF)).copy()
    mtx = np.zeros((PT, 4, 51), np.float32)
    mty = np.zeros((PT, 4, 51), np.float32)
    mtc = np.zeros((PT, 4, 51), np.float32)
    for ch in range(4):
        for p in range(PT):
            jk = ch * PT + p
            if jk < JK:
                j = jk // K
                mtx[p, ch, j] = 1.0
                mty[p, ch, 17 + j] = 1.0
                mtc[p, ch, 34 + j] = 1.0
    c["maskTx"] = mtx.reshape(PT, 4 * 51).copy()
    c["maskTy"] = mty.reshape(PT, 4 * 51).copy()
    c["maskTc"] = mtc.reshape(PT, 4 * 51).copy()
    c["wzero"] = np.zeros((CAUG, JKP), np.float32)
    c["czero"] = np.zeros((2, JKP), np.float32)
    c["cgidx"] = np.broadcast_to(
        np.repeat(np.arange(NSLAB, dtype=np.float32), 8), (J, NSLAB * 8)).copy()
    return c


def _prep_shards(poses, heat, off):
    consts = _build_consts()
    heat_pad = np.full((J, H + 4, W + 4), -1.0, np.float32)
    heat_pad[:, 2:-2, 2:-2] = heat
    in_maps = []
    for core in range(NCORES):
        r0 = core * RPC
        lo = core * NPAD
        ps = poses[min(lo, len(poses)):min(lo + NPAD, len(poses))]
        pa = np.zeros((NPAD, CAUG), np.float32)
        if len(ps):
            pa[:len(ps), 0:17] = ps[:, 0::2]
            pa[:len(ps), 17:34] = ps[:, 1::2]
        pa[:, 34] = 1.0
        slab = np.full((NTILE_H * PT, SLABW, RW), -1.0, np.float32)
        for cg in range(NSLAB):
            tile_i, cg_l = divmod(cg, SPT)
            c0 = cg * SW
            ncol = min(SLABW, W + 4 - c0)
            blk = heat_pad[:, r0:r0 + RW, c0:c0 + ncol]       # [J, 68, ncol]
            for j in range(J):
                p = tile_i * PT + cg_l * J + j
                slab[p, :ncol, :] = blk[j].T
        m = {
            "poses": pa,
            "heat": slab.reshape(NTILE_H * PT, SLABW * RW),
            "offs": np.ascontiguousarray(
                off[:, :, r0:r0 + RPC, :].transpose(2, 3, 0, 1)
            ).reshape(OFFSZ, 1),
            "coreconst": np.broadcast_to(
                np.array([r0 * W, r0], np.float32), (J, 2)).copy(),
        }
        m.update(consts)
        in_maps.append(m)
    return in_maps


def _fixup(out_full, cnt, cand, poses):
    """Recompute sites where the one-hot matched != 1 candidate, exactly."""
    hx = cand[:, 0:K]
    hy = cand[:, K:2 * K]
    bad = np.argwhere(np.abs(cnt - 1.0) > 0.25)
    for n, j in bad:
        if n >= len(poses):
            continue
        px = np.float32(poses[n, 2 * j])
        py = np.float32(poses[n, 2 * j + 1])
        dx = (px - hx[j]).astype(np.float32)
        dy = (py - hy[j]).astype(np.float32)
        d2 = (dx * dx + dy * dy).astype(np.float32)
        kk = int(np.argmin(d2))
        out_full[n, 2 * j] = hx[j, kk]
        out_full[n, 2 * j + 1] = hy[j, kk]
    return out_full


def kernel(poses, heat_pred, offset_pred, stride):
    from concourse.bass_utils import run_bass_kernel_spmd

    poses = np.asarray(poses, dtype=np.float32)
    heat_pred = np.asarray(heat_pred, dtype=np.float32)
    offset_pred = np.asarray(offset_pred, dtype=np.float32)
    stride_v = int(np.asarray(stride).reshape(-1)[0]) if np.ndim(stride) else int(stride)

    key = ("prog", stride_v)
    if key not in _CACHE:
        _CACHE[key] = _build_program(stride_v)
    nc = _CACHE[key]

    in_maps = _prep_shards(poses, heat_pred, offset_pred)
    r = run_bass_kernel_spmd(nc, in_maps, list(range(NCORES)))
    global LAST_EXEC_NS
    LAST_EXEC_NS = r.exec_time_ns
    res = r.results

    outs = []
    cand = np.asarray(res[0]["cand"], dtype=np.float32)
    for core in range(NCORES):
        o = np.asarray(res[core]["out"], dtype=np.float32)   # [NPAD, 51]
        outs.append(o)
    N = len(poses)
    full = np.zeros((N, 2 * J), np.float32)
    cnt_full = np.zeros((N, J), np.float32)
    for core in range(NCORES):
        lo = core * NPAD
        hi = min(lo + NPAD, N)
        if hi <= lo:
            break
        o = outs[core][:hi - lo]
        full[lo:hi, 0::2] = o[:, 0:17]
        full[lo:hi, 1::2] = o[:, 17:34]
        cnt_full[lo:hi] = o[:, 34:51]
    full = _fixup(full, cnt_full, cand, poses)
    return full


# revision 71
# speedup vs baseline: 1.0248x; 1.0248x over previous
"""Trainium2 Bass kernel for nn_PointSetAnchorPoseHead (NMS pose decode).

Runs on 8 NeuronCores via run_bass_kernel_spmd. See bottom for host glue.

Algorithm (per core, SPMD):
  heat stage: rows sharded 64/core (+2 halo). 5x5 maxpool via shifted-max
  cascades on (col,row)-in-free layout, work split across DVE and Pool
  engines by column ranges; exact key packing: for values v>t (t=1-2^-11)
  key = (v-t)*2^35 + (2047-slabidx), a 24-bit exact f32 int. max8 per
  (joint, 25-col slab) -> per-core top-16. The core then decodes ALL its
  slab candidates, eq-match-selects gpos for its top-16, gathers the
  matching offsets from its offset shard (one indirect DMA), and packs
  (key | gpos | offx | offy) into a single AllGather. After the gather
  every core replicates the merge: top-30 keys via max8+match_replace,
  then a partition-spread eq-match select pulls (gpos, offx, offy) for
  the 30 winners. Only ONE collective total (no AllReduce).
  pose stage: 98 tiles of 128 poses, software-pipelined 4 deep:
  score = |c|^2 - 2 q.c in one fp32 PE matmul (block-diag W built on
  device; pose transposes precomputed during the heat phase on idle
  PE/Act). rmin on DVE, one-hot is_lt on Pool, 4 bf16 PE transposes into
  one PSUM tile, single Act copy, 3-way bf16-split gather matmul, Pool
  copy to a 4-tile staging buffer, batched output DMA. Host recomputes
  the few count!=1 sites exactly (reference f32 arithmetic).
"""

import numpy as np

J = 17
K = 30
H = 512
W = 512
NCORES = 8
RPC = H // NCORES          # 64 rows per core
PT = 128
NT = 98
NPAD = PT * NT             # 12544
CAUG = 36                  # x17, y17, 1, 0
JK = J * K                 # 510
JKP = 512
SW = 25                    # slab width; 21 slabs
NSLAB = 21
SLABW = SW + 4             # stored cols (2 halo each side, 29)
RW = RPC + 4               # stored rows (68)
SLABF = SW * RPC           # 1600 owned cells
SPT = 7                    # slabs per heat partition-tile
NTILE_H = 3
THRESH_T = float(1.0 - 2.0 ** -11)
KEYSCALE = float(2.0 ** 35)
SCORE_THRESH = 32.0
NEG = -1.0e30
OFFSZ = J * 2 * RPC * W

_CACHE = {}
LAST_EXEC_NS = None


# --------------------------------------------------------------------------
# device program
# --------------------------------------------------------------------------
def _build_program(stride, ntiles=NT, debug=False):
    import concourse.bass as bass
    import concourse.bacc as bacc
    import concourse.mybir as mybir
    from concourse import tile

    dt = mybir.dt
    Alu = mybir.AluOpType
    Ax = mybir.AxisListType
    nc = bacc.Bacc(None)

    def din(name, shape, dtype=dt.float32):
        return nc.declare_dram_parameter(name, list(shape), dtype, isOutput=False)

    poses_d = din("poses", [NPAD, CAUG])
    heat_d = din("heat", [NTILE_H * PT, SLABW * RW])
    offs_d = din("offs", [OFFSZ, 1])
    cconst_d = din("coreconst", [J, 2])
    identf_d = din("identf", [PT, PT])
    identb_d = din("identb", [PT, PT], dt.bfloat16)
    rev_d = din("revconst", [PT, SLABF])
    mtx_d = din("maskTx", [PT, 4 * 51])
    mty_d = din("maskTy", [PT, 4 * 51])
    mtc_d = din("maskTc", [PT, 4 * 51])
    cgidx_d = din("cgidx", [J, NSLAB * 8])
    wz_d = din("wzero", [CAUG, JKP])
    cz_d = din("czero", [2, JKP])

    out_d = nc.declare_dram_parameter("out", [NPAD, 51], dt.float32, isOutput=True)
    cand_d = nc.declare_dram_parameter("cand", [J, PT], dt.float32, isOutput=True)
    if debug:
        dbg_d = nc.declare_dram_parameter("dbg", [J, 1024], dt.float32,
                                          isOutput=True)
        dbg2_d = nc.declare_dram_parameter("dbg2", [102, 64], dt.float32,
                                           isOutput=True)

    with tile.TileContext(nc) as tc:
        with (
            tc.tile_pool(name="const", bufs=1) as cpool,
            tc.tile_pool(name="heatp", bufs=2) as hpool,
            tc.tile_pool(name="work", bufs=1) as wpool,
            tc.tile_pool(name="small", bufs=1) as spool,
            tc.tile_pool(name="pose", bufs=1) as ppool,
            tc.tile_pool(name="loop", bufs=2) as lpool,
            tc.tile_pool(name="merge", bufs=1) as mpool,
            tc.tile_pool(name="psA", bufs=2, space="PSUM") as psA,
            tc.tile_pool(name="psB", bufs=2, space="PSUM") as psB,
            tc.tile_pool(name="dram", bufs=1, space="DRAM") as dpool,
        ):
            # ---------- heat tile DMAs first (don't sit behind poses DMA) ----
            hx_tiles = []
            for ti in range(2):
                hx = hpool.tile([PT, SLABW * RW], dt.float32, tag="heat")
                if ti == 0:
                    nc.sync.dma_start(hx[:, 0:15 * RW],
                                      heat_d[0:PT, 0:15 * RW])
                    nc.sync.dma_start(hx[:, 15 * RW:],
                                      heat_d[0:PT, 15 * RW:])
                else:
                    nc.sync.dma_start(hx[:], heat_d[ti * PT:(ti + 1) * PT, :])
                hx_tiles.append(hx)

            # ---------- constants ----------
            identf = cpool.tile([PT, PT], dt.float32)
            nc.sync.dma_start(identf[:], identf_d[:])
            identb = cpool.tile([PT, PT], dt.bfloat16)
            nc.sync.dma_start(identb[:], identb_d[:])
            rev = cpool.tile([PT, SLABF], dt.float32)
            nc.sync.dma_start(rev[:], rev_d[:])
            mtx = cpool.tile([PT, 4 * 51], dt.float32)
            nc.sync.dma_start(mtx[:], mtx_d[:])
            mty = cpool.tile([PT, 4 * 51], dt.float32)
            nc.sync.dma_start(mty[:], mty_d[:])
            mtc = cpool.tile([PT, 4 * 51], dt.float32)
            nc.sync.dma_start(mtc[:], mtc_d[:])
            cconst = cpool.tile([J, 2], dt.float32)
            nc.sync.dma_start(cconst[:], cconst_d[:])
            cg_f = cpool.tile([J, NSLAB * 8], dt.float32)
            nc.sync.dma_start(cg_f[:], cgidx_d[:])
            # spread constants used later (ready immediately)
            r0b = cpool.tile([J * 6, 1], dt.float32)
            nc.sync.dma_start(
                bass.AP(r0b.tensor, 0, [[1, J * 6], [1, 1]]),
                bass.AP(cconst.tensor, 1, [[2, J], [0, 6], [1, 1]]))
            jr_i = cpool.tile([J, 1], dt.int32)
            nc.gpsimd.iota(jr_i[:], pattern=[[0, 1]], base=0, channel_multiplier=1)
            jrowf = cpool.tile([J, 1], dt.float32)
            nc.vector.tensor_copy(jrowf[:], jr_i[:])
            j2 = cpool.tile([J, 1], dt.float32)
            nc.vector.tensor_scalar_mul(j2[:], jrowf[:], 2.0)
            j2b = cpool.tile([J * 6, 1], dt.float32)
            nc.sync.dma_start(
                bass.AP(j2b.tensor, 0, [[1, J * 6], [1, 1]]),
                bass.AP(j2.tensor, 0, [[1, J], [0, 6], [1, 1]]))

            posesb = ppool.tile([PT, NT * CAUG], dt.float32)
            nc.sync.dma_start(
                posesb[:],
                bass.AP(poses_d[:].tensor, 0,
                        [[CAUG, PT], [PT * CAUG, NT], [1, CAUG]]))

            # ---------- pose transposes precomputed on idle PE/Act ----------
            NB2 = (ntiles + 1) // 2
            posesT = ppool.tile([64 + CAUG, NB2 * PT], dt.float32)

            def posesT_slice(t):
                ch, blk = t % 2, t // 2
                return posesT[ch * 64:ch * 64 + CAUG,
                              blk * PT:(blk + 1) * PT]

            for t in range(ntiles):
                pT_ps = psB.tile([CAUG, PT], dt.float32, tag="psb", bufs=2)
                nc.tensor.transpose(pT_ps[:], posesb[:, t * CAUG:(t + 1) * CAUG],
                                    identf[:])
                nc.scalar.copy(posesT_slice(t), pT_ps[:])

            # ---------- heat stage (DVE/Pool split by slab columns) ----------
            def ap(t, coff, roff, ccnt, rcnt, rw):
                return bass.AP(t.tensor, coff * rw + roff,
                               [[t.shape[1], PT], [rw, ccnt], [1, rcnt]])

            def split_tt(op, out_t, rw_o, in0_t, co0, ro0, rw0,
                         in1_t, co1, ro1, rw1, ncols, rcnt, dcols):
                nc.vector.tensor_tensor(
                    out=ap(out_t, 0, 0, ncols, rcnt, rw_o),
                    in0=ap(in0_t, co0, ro0, ncols, rcnt, rw0),
                    in1=ap(in1_t, co1, ro1, ncols, rcnt, rw1), op=op)

            kall_ps = psA.tile([J, NSLAB * 8], dt.float32, tag="gps", bufs=2)
            kall = spool.tile([J, NSLAB * 8], dt.float32)
            for ti in range(NTILE_H):
                if ti < 2:
                    hx = hx_tiles[ti]
                else:
                    hx = hpool.tile([PT, SLABW * RW], dt.float32, tag="heat")
                    nc.sync.dma_start(hx[:], heat_d[ti * PT:(ti + 1) * PT, :])

                m1 = wpool.tile([PT, SLABW * 67], dt.float32, tag="m1")
                if ti == 0:
                    for c0, cn in ((0, 15), (15, SLABW - 15)):
                        nc.vector.tensor_tensor(
                            out=ap(m1, c0, 0, cn, 67, 67),
                            in0=ap(hx, c0, 0, cn, 67, RW),
                            in1=ap(hx, c0, 1, cn, 67, RW), op=Alu.max)
                else:
                    split_tt(Alu.max, m1, 67, hx, 0, 0, RW, hx, 0, 1, RW,
                             SLABW, 67, 17)
                m2 = wpool.tile([PT, SLABW * 65], dt.float32, tag="m2")
                split_tt(Alu.max, m2, 65, m1, 0, 0, 67, m1, 0, 2, 67,
                         SLABW, 65, 17)
                w5r = wpool.tile([PT, SLABW * RPC], dt.float32, tag="w5r")
                split_tt(Alu.max, w5r, RPC, m2, 0, 0, 65, hx, 0, 4, RW,
                         SLABW, RPC, 17)
                n1 = wpool.tile([PT, 28 * RPC], dt.float32, tag="n1")
                split_tt(Alu.max, n1, RPC, w5r, 0, 0, RPC, w5r, 1, 0, RPC,
                         28, RPC, 16)
                n2 = wpool.tile([PT, 26 * RPC], dt.float32, tag="n2")
                split_tt(Alu.max, n2, RPC, n1, 0, 0, RPC, n1, 2, 0, RPC,
                         26, RPC, 15)
                w55 = wpool.tile([PT, SW * RPC], dt.float32, tag="w55")
                split_tt(Alu.max, w55, RPC, n2, 0, 0, RPC, w5r, 4, 0, RPC,
                         SW, RPC, 13)
                eq = wpool.tile([PT, SW * RPC], dt.float32, tag="n1")
                split_tt(Alu.is_equal, eq, RPC, hx, 2, 2, RW, w55, 0, 0, RPC,
                         SW, RPC, 13)
                r1 = wpool.tile([PT, SW * RPC], dt.float32, tag="m2")
                keyt = wpool.tile([PT, SLABF], dt.float32, tag="m1")
                nc.vector.scalar_tensor_tensor(
                    out=ap(r1, 0, 0, SW, RPC, RPC),
                    in0=ap(hx, 2, 2, SW, RPC, RW), scalar=-THRESH_T,
                    in1=ap(eq, 0, 0, SW, RPC, RPC),
                    op0=Alu.add, op1=Alu.mult)
                nc.vector.scalar_tensor_tensor(
                    out=keyt[:], in0=r1[:], scalar=KEYSCALE, in1=rev[:],
                    op0=Alu.mult, op1=Alu.add)
                k8t = wpool.tile([PT, 8], dt.float32, tag="k8t")
                nc.vector.max(k8t[:], keyt[:])
                # regroup k8t [cgl*17+j, v] -> kall[j, (ti*7+cgl)*8+v] with
                # one-hot selector matmuls on the idle PE (exact for 0/1 wts)
                for cgl in range(SPT):
                    nc.tensor.matmul(
                        kall_ps[:, (ti * SPT + cgl) * 8:(ti * SPT + cgl + 1) * 8],
                        identf[0:SPT * J, cgl * J:(cgl + 1) * J],
                        k8t[0:SPT * J, :], start=True, stop=True)

            # ---------- per-core top-16 ----------
            nc.scalar.copy(kall[:], kall_ps[:])
            kwork = spool.tile([J, NSLAB * 8], dt.float32)
            nc.vector.tensor_copy(kwork[:], kall[:])
            key16p = spool.tile([J, 18], dt.float32)
            nc.vector.memset(key16p[:], NEG)
            key16 = key16p[:, 0:16]
            nc.vector.max(key16p[:, 0:8], kwork[:])
            nc.vector.match_replace(kwork[:], key16p[:, 0:8], kwork[:], NEG)
            nc.vector.max(key16p[:, 8:16], kwork[:])

            # decode all local per-slab candidates -> gposall [17, 168]
            ki = spool.tile([J, NSLAB * 8], dt.int32)
            kclamp = spool.tile([J, NSLAB * 8], dt.float32)
            nc.vector.tensor_scalar_max(kclamp[:], kall[:], 0.0)
            nc.vector.tensor_copy(ki[:], kclamp[:])
            s11 = spool.tile([J, NSLAB * 8], dt.int32)
            nc.vector.tensor_scalar(out=s11[:], in0=ki[:], scalar1=2047,
                                    scalar2=None, op0=Alu.bitwise_and)
            nc.vector.tensor_scalar(out=s11[:], in0=s11[:], scalar1=-2047,
                                    scalar2=-1, op0=Alu.add, op1=Alu.mult)
            ci = spool.tile([J, NSLAB * 8], dt.int32)
            nc.vector.tensor_scalar(out=ci[:], in0=s11[:], scalar1=6,
                                    scalar2=None, op0=Alu.arith_shift_right)
            ri = spool.tile([J, NSLAB * 8], dt.int32)
            nc.vector.tensor_scalar(out=ri[:], in0=s11[:], scalar1=RPC - 1,
                                    scalar2=None, op0=Alu.bitwise_and)
            cf = spool.tile([J, NSLAB * 8], dt.float32)
            nc.vector.tensor_copy(cf[:], ci[:])
            rf = spool.tile([J, NSLAB * 8], dt.float32)
            nc.vector.tensor_copy(rf[:], ri[:])
            gcol = spool.tile([J, NSLAB * 8], dt.float32)
            nc.vector.scalar_tensor_tensor(out=gcol[:], in0=cg_f[:], scalar=float(SW),
                                           in1=cf[:], op0=Alu.mult, op1=Alu.add)
            gposall = spool.tile([J, NSLAB * 8], dt.float32)
            nc.vector.scalar_tensor_tensor(out=gposall[:], in0=rf[:], scalar=float(W),
                                           in1=gcol[:], op0=Alu.mult, op1=Alu.add)
            nc.vector.tensor_scalar(out=gposall[:], in0=gposall[:],
                                    scalar1=cconst[:, 0:1], scalar2=None, op0=Alu.add)

            # ---------- select gpos for top-16 (partition-spread match) -----
            # p = j*6+kk handles keys 3kk..3kk+2; each vs all 168 candidates
            NSP = NSLAB * 8            # 168
            kall_sp = spool.tile([J * 6, NSP], dt.float32)
            nc.sync.dma_start(
                kall_sp[:],
                bass.AP(kall.tensor, 0, [[NSP, J], [0, 6], [1, NSP]]))
            k16_sp = spool.tile([J * 6, 3], dt.float32)
            nc.sync.dma_start(
                bass.AP(k16_sp.tensor, 0, [[3, J * 6], [1, 3]]),
                bass.AP(key16p.tensor, 0, [[18, J], [1, 18]]))
            gpos_sp = spool.tile([J * 6, NSP], dt.float32)
            nc.sync.dma_start(
                gpos_sp[:],
                bass.AP(gposall.tensor, 0, [[NSP, J], [0, 6], [1, NSP]]))
            eq16 = spool.tile([J * 6, 3 * NSP], dt.float32)
            nc.vector.tensor_tensor(
                out=bass.AP(eq16.tensor, 0, [[3 * NSP, J * 6], [NSP, 3], [1, NSP]]),
                in0=bass.AP(k16_sp.tensor, 0, [[3, J * 6], [1, 3], [0, NSP]]),
                in1=bass.AP(kall_sp.tensor, 0, [[NSP, J * 6], [0, 3], [1, NSP]]),
                op=Alu.is_equal)
            nc.vector.tensor_tensor(
                out=eq16[:], in0=eq16[:],
                in1=bass.AP(gpos_sp.tensor, 0, [[NSP, J * 6], [0, 3], [1, NSP]]),
                op=Alu.mult)
            g16_sp = spool.tile([J * 6, 3], dt.float32)
            nc.vector.tensor_reduce(
                g16_sp[:],
                bass.AP(eq16.tensor, 0, [[3 * NSP, J * 6], [NSP, 3], [1, NSP]]),
                axis=Ax.X, op=Alu.add)
            # decode/idx in spread layout [102, 3]; unmatched slots give
            # gpos 0 from the eq-sum, and fake keys never reach the top-30
            gq = spool.tile([J * 6, 3], dt.int32)
            nc.vector.tensor_copy(gq[:], g16_sp[:])
            gqr = spool.tile([J * 6, 3], dt.int32)
            nc.vector.tensor_scalar(out=gqr[:], in0=gq[:], scalar1=9,
                                    scalar2=None, op0=Alu.arith_shift_right)
            rowf = spool.tile([J * 6, 3], dt.float32)
            nc.vector.tensor_copy(rowf[:], gqr[:])
            gqc = spool.tile([J * 6, 3], dt.int32)
            nc.vector.tensor_scalar(out=gqc[:], in0=gq[:], scalar1=W - 1,
                                    scalar2=None, op0=Alu.bitwise_and)
            colf = spool.tile([J * 6, 3], dt.float32)
            nc.vector.tensor_copy(colf[:], gqc[:])
            lrow = spool.tile([J * 6, 3], dt.float32)
            nc.vector.tensor_scalar(out=lrow[:], in0=rowf[:],
                                    scalar1=r0b[:, 0:1], scalar2=None,
                                    op0=Alu.subtract)
            nc.vector.tensor_scalar(out=lrow[:], in0=lrow[:], scalar1=0.0,
                                    scalar2=float(RPC - 1), op0=Alu.max, op1=Alu.min)
            # offs shard is transposed on host to [lrow, col, j, 2]:
            # site addr = (lrow*W + col)*2J + 2j; gathers (offx, offy) adjacent
            idx_sp = spool.tile([J * 6, 3], dt.float32)
            nc.vector.scalar_tensor_tensor(out=idx_sp[:], in0=lrow[:],
                                           scalar=float(W), in1=colf[:],
                                           op0=Alu.mult, op1=Alu.add)
            nc.vector.scalar_tensor_tensor(out=idx_sp[:], in0=idx_sp[:],
                                           scalar=float(2 * J),
                                           in1=bass.AP(j2b.tensor, 0,
                                                       [[1, J * 6], [1, 1], [0, 3]]),
                                           op0=Alu.mult, op1=Alu.add)
            idxp = spool.tile([J * 6, 3], dt.int32)
            nc.vector.tensor_copy(idxp[:], idx_sp[:])
            offs_g = spool.tile([J * 6, 6], dt.float32)
            nc.vector.memset(offs_g[:], 0.0)
            for c in range(3):
                nc.gpsimd.indirect_dma_start(
                    out=offs_g[:, 2 * c:2 * c + 2], out_offset=None,
                    in_=offs_d[:],
                    in_offset=bass.IndirectOffsetOnAxis(ap=idxp[:, c:c + 1],
                                                        axis=0),
                    bounds_check=OFFSZ - 2, oob_is_err=False)
            # ---------- single AllGather: key18 | gpos18 | offx18 | offy18 --
            ag_in = dpool.tile([J, 72], dt.float32)
            ag_out = dpool.tile([NCORES * J, 72], dt.float32)
            nc.scalar.dma_start(ag_in[:, 0:18], key16p[:])
            nc.scalar.dma_start(
                bass.AP(ag_in[:].tensor, 18, [[72, J], [1, 18]]),
                bass.AP(g16_sp.tensor, 0, [[3, J * 6], [1, 3]]))
            nc.scalar.dma_start(
                bass.AP(ag_in[:].tensor, 36, [[72, J], [1, 18]]),
                bass.AP(offs_g.tensor, 0, [[6, J * 6], [2, 3]]))
            nc.sync.dma_start(
                bass.AP(ag_in[:].tensor, 54, [[72, J], [1, 18]]),
                bass.AP(offs_g.tensor, 1, [[6, J * 6], [2, 3]]))
            nc.gpsimd.collective_compute(
                "AllGather", Alu.bypass,
                replica_groups=[list(range(NCORES))],
                ins=[ag_in[:]], outs=[ag_out[:]])

            NCF = NCORES * 18          # 144
            NC16 = NCF
            pools = {}
            for fi, nm in enumerate(("kpool", "gpool", "oxpool", "oypool")):
                pt = mpool.tile([J, NCF], dt.float32, name=f"pt_{nm}")
                nc.sync.dma_start(
                    pt[:],
                    bass.AP(ag_out.tensor, fi * 18,
                            [[72, J], [J * 72, NCORES], [1, 18]]))
                pools[nm] = pt
            sp = {}
            for nm in ("kpool", "gpool", "oxpool", "oypool"):
                t = mpool.tile([J * 6, NC16], dt.float32, name=f"sp_{nm}")
                nc.sync.dma_start(
                    bass.AP(t.tensor, 0, [[NC16, J * 6], [1, NC16]]),
                    bass.AP(pools[nm].tensor, 0, [[NC16, J], [0, 6], [1, NC16]]))
                sp[nm] = t

            # ---------- merge: top-32 keys (use first 30) ----------
            kmw = mpool.tile([J, NCF], dt.float32)
            nc.vector.tensor_copy(kmw[:], pools["kpool"][:])
            fkeys = mpool.tile([J, 32], dt.float32)
            for r in range(4):
                nc.vector.max(fkeys[:, r * 8:(r + 1) * 8], kmw[:])
                if r < 3:
                    nc.vector.match_replace(kmw[:], fkeys[:, r * 8:(r + 1) * 8],
                                            kmw[:], NEG)

            # ---------- spread select of (gpos, offx, offy) for 30 winners --
            # p = j*6+kk handles final keys kk*5..kk*5+4, each vs 144 cands
            fk_sp = mpool.tile([J * 6, 5], dt.float32)
            nc.scalar.dma_start(
                bass.AP(fk_sp.tensor, 0, [[5, J * 6], [1, 5]]),
                bass.AP(fkeys.tensor, 0, [[32, J], [1, 30]]))
            eqf = mpool.tile([J * 6, 5 * NC16], dt.float32)
            nc.vector.tensor_tensor(
                out=bass.AP(eqf.tensor, 0, [[5 * NC16, J * 6], [NC16, 5], [1, NC16]]),
                in0=bass.AP(fk_sp.tensor, 0, [[5, J * 6], [1, 5], [0, NC16]]),
                in1=bass.AP(sp["kpool"].tensor, 0,
                            [[NC16, J * 6], [0, 5], [1, NC16]]),
                op=Alu.is_equal)
            sel3 = mpool.tile([J * 6, 15], dt.float32)
            prods = [mpool.tile([J * 6, 5 * NC16], dt.float32, name="prodA"),
                     mpool.tile([J * 6, 5 * NC16], dt.float32, name="prodB")]
            for pi, (nm, eng) in enumerate((("gpool", nc.vector),
                                            ("oxpool", nc.vector),
                                            ("oypool", nc.vector))):
                prod = prods[pi % 2]
                eng.tensor_tensor(
                    out=prod[:], in0=eqf[:],
                    in1=bass.AP(sp[nm].tensor, 0,
                                [[NC16, J * 6], [0, 5], [1, NC16]]),
                    op=Alu.mult)
                nc.vector.tensor_reduce(
                    sel3[:, pi * 5:(pi + 1) * 5],
                    bass.AP(prod.tensor, 0, [[5 * NC16, J * 6], [NC16, 5], [1, NC16]]),
                    axis=Ax.X, op=Alu.add)
            # decode directly in spread layout [102, 5] (p = j*6+kk)

            # final candidate coords (reference arithmetic: stride*(x+off))
            yq = spool.tile([J * 6, 5], dt.int32)
            nc.vector.tensor_copy(yq[:], sel3[:, 0:5])
            yqs = spool.tile([J * 6, 5], dt.int32)
            nc.vector.tensor_scalar(out=yqs[:], in0=yq[:], scalar1=9,
                                    scalar2=None, op0=Alu.arith_shift_right)
            yf = spool.tile([J * 6, 5], dt.float32)
            nc.vector.tensor_copy(yf[:], yqs[:])
            xq = spool.tile([J * 6, 5], dt.int32)
            nc.vector.tensor_scalar(out=xq[:], in0=yq[:], scalar1=W - 1,
                                    scalar2=None, op0=Alu.bitwise_and)
            xf = spool.tile([J * 6, 5], dt.float32)
            nc.vector.tensor_copy(xf[:], xq[:])
            hxc = spool.tile([J * 6, 5], dt.float32)
            hyc = spool.tile([J * 6, 5], dt.float32)
            nc.vector.tensor_tensor(out=hxc[:], in0=xf[:], in1=sel3[:, 5:10],
                                    op=Alu.add)
            nc.vector.tensor_scalar_mul(hxc[:], hxc[:], float(stride))
            nc.vector.tensor_tensor(out=hyc[:], in0=yf[:], in1=sel3[:, 10:15],
                                    op=Alu.add)
            nc.vector.tensor_scalar_mul(hyc[:], hyc[:], float(stride))
            # cand output: DRAM-side reorder [j*6+kk, q] -> cand[j, kk*5+q]
            for sap, c0 in ((hxc[:], 0), (hyc[:], K), (sel3[:, 0:5], 2 * K)):
                nc.gpsimd.dma_start(
                    bass.AP(cand_d[:].tensor, c0, [[PT, J], [5, 6], [1, 5]]),
                    sap)

            if debug:
                nc.sync.dma_start(dbg_d[:, 0:168], kall[:])
                nc.sync.dma_start(dbg_d[:, 168:336], gposall[:])
                nc.sync.dma_start(dbg_d[:, 336:354], key16p[:])
                nc.sync.dma_start(
                    bass.AP(dbg_d[:].tensor, 354, [[1024, J], [1, 18]]),
                    bass.AP(g16_sp.tensor, 0, [[3, J * 6], [1, 3]]))
                nc.sync.dma_start(dbg_d[:, 388:532], pools["kpool"][:])
                nc.sync.dma_start(dbg_d[:, 532:676], pools["gpool"][:])
                nc.sync.dma_start(dbg_d[:, 676:820], pools["oxpool"][:])
                nc.sync.dma_start(dbg_d[:, 836:868], fkeys[:])
                nc.sync.dma_start(dbg2_d[:, 0:15], sel3[:])
                nc.sync.dma_start(dbg2_d[:, 15:20], fk_sp[:])
                nc.sync.dma_start(dbg2_d[:, 20:25], hxc[:])

            # ---------- score matrix W [36, 512] assembled in DRAM ----------
            # wz_d is host-zeroed; write the block-diagonal parts:
            #   row j cols j*30+k = -2cx; row 17+j = -2cy; row 34 = cx^2+cy^2
            m2x = spool.tile([J * 6, 5], dt.float32)
            nc.vector.tensor_scalar_mul(m2x[:], hxc[:], -2.0)
            m2y = spool.tile([J * 6, 5], dt.float32)
            nc.vector.tensor_scalar_mul(m2y[:], hyc[:], -2.0)
            cx2 = spool.tile([J * 6, 5], dt.float32)
            nc.vector.tensor_tensor(out=cx2[:], in0=hxc[:], in1=hxc[:], op=Alu.mult)
            c2s = spool.tile([J * 6, 5], dt.float32)
            nc.vector.scalar_tensor_tensor(out=c2s[:], in0=hyc[:], scalar=1.0,
                                           in1=hyc[:], op0=Alu.mult, op1=Alu.mult)
            nc.vector.tensor_tensor(out=c2s[:], in0=c2s[:], in1=cx2[:], op=Alu.add)
            nc.sync.dma_start(
                bass.AP(wz_d[:].tensor, 0, [[JKP + K, J], [5, 6], [1, 5]]),
                m2x[:])
            nc.scalar.dma_start(
                bass.AP(wz_d[:].tensor, J * JKP, [[JKP + K, J], [5, 6], [1, 5]]),
                m2y[:])
            nc.sync.dma_start(
                bass.AP(wz_d[:].tensor, 34 * JKP, [[K, J], [5, 6], [1, 5]]),
                c2s[:])
            wmat = spool.tile([64 + CAUG, JKP], dt.float32)
            nc.scalar.dma_start(wmat[0:CAUG, :], wz_d[:])
            nc.sync.dma_start(wmat[64:64 + CAUG, :], wz_d[:])
            # candidate coords chunked [128, 4] via host-zeroed DRAM bounce
            nc.gpsimd.dma_start(
                bass.AP(cz_d[:].tensor, 0, [[K, J], [5, 6], [1, 5]]), hxc[:])
            nc.gpsimd.dma_start(
                bass.AP(cz_d[:].tensor, JKP, [[K, J], [5, 6], [1, 5]]), hyc[:])

            # ---------- gather table T (bf16 3-split) ----------
            cxP = spool.tile([PT, 4], dt.float32)
            cyP = spool.tile([PT, 4], dt.float32)
            nc.sync.dma_start(
                cxP[:], bass.AP(cz_d[:].tensor, 0, [[1, PT], [PT, 4]]))
            nc.sync.dma_start(
                cyP[:], bass.AP(cz_d[:].tensor, JKP, [[1, PT], [PT, 4]]))
            t_y = spool.tile([PT, 4 * 51], dt.float32)
            tfull = spool.tile([PT, 4 * 51], dt.float32)
            for c in range(4):
                nc.vector.scalar_tensor_tensor(
                    out=t_y[:, c * 51:(c + 1) * 51],
                    in0=mty[:, c * 51:(c + 1) * 51], scalar=cyP[:, c:c + 1],
                    in1=mtc[:, c * 51:(c + 1) * 51], op0=Alu.mult, op1=Alu.add)
                nc.vector.scalar_tensor_tensor(
                    out=tfull[:, c * 51:(c + 1) * 51],
                    in0=mtx[:, c * 51:(c + 1) * 51], scalar=cxP[:, c:c + 1],
                    in1=t_y[:, c * 51:(c + 1) * 51], op0=Alu.mult, op1=Alu.add)
            t_hi = spool.tile([PT, 4 * 51], dt.bfloat16)
            t_mid = spool.tile([PT, 4 * 51], dt.bfloat16)
            t_lo = spool.tile([PT, 4 * 51], dt.bfloat16)
            tr1 = spool.tile([PT, 4 * 51], dt.float32)
            tr1b = spool.tile([PT, 4 * 51], dt.float32)
            nc.vector.tensor_copy(t_hi[:], tfull[:])
            nc.vector.tensor_copy(tr1b[:], t_hi[:])
            nc.vector.tensor_tensor(out=tr1[:], in0=tfull[:], in1=tr1b[:],
                                    op=Alu.subtract)
            nc.vector.tensor_copy(t_mid[:], tr1[:])
            nc.vector.tensor_copy(tr1b[:], t_mid[:])
            nc.vector.tensor_tensor(out=tr1[:], in0=tr1[:], in1=tr1b[:],
                                    op=Alu.subtract)
            nc.vector.tensor_copy(t_lo[:], tr1[:])

            # ---------- pose loop (software-pipelined, 5 stages) ----------
            thrc = spool.tile([PT, 1], dt.float32)
            nc.vector.memset(thrc[:], SCORE_THRESH)
            score_t = {}
            oh_t = {}
            ohT_t = {}
            gst_ref = [None]

            def st_score(t):
                s = psA.tile([PT, JKP], dt.float32, tag="score", bufs=3)
                ch = t % 2
                nc.tensor.matmul(s[:], posesT_slice(t),
                                 wmat[ch * 64:ch * 64 + CAUG, :],
                                 start=True, stop=True)
                score_t[t] = s

            rminp_t = {}

            def st_rmin(t):
                s = score_t[t]
                sc3 = bass.AP(s.tensor, 0, [[JKP, PT], [K, J], [1, K]])
                rmin = lpool.tile([PT, J], dt.float32, tag="rmin")
                nc.vector.tensor_reduce(rmin[:], sc3, axis=Ax.X, op=Alu.min)
                rminp = lpool.tile([PT, J], dt.float32, tag="rminp")
                nc.scalar.add(rminp[:], rmin[:], thrc[:, 0:1])
                rminp_t[t] = rminp

            def st_islt(t):
                s = score_t.pop(t)
                rminp = rminp_t.pop(t)
                sc3 = bass.AP(s.tensor, 0, [[JKP, PT], [K, J], [1, K]])
                oh = lpool.tile([PT, JKP], dt.bfloat16, tag="oh")
                nc.scalar.memzero(oh[:, JK:JKP])
                rb = bass.AP(rminp.tensor, 0, [[J, PT], [1, J], [0, K]])
                nc.vector.tensor_tensor(
                    out=bass.AP(oh.tensor, 0, [[JKP, PT], [K, J], [1, K]]),
                    in0=sc3, in1=rb, op=Alu.is_lt)
                oh_t[t] = oh

            def st_transp(t):
                oh = oh_t.pop(t)
                ohT_ps = psA.tile([PT, JKP], dt.bfloat16, tag="ohTps", bufs=1)
                for c in range(4):
                    nc.tensor.transpose(ohT_ps[:, c * PT:(c + 1) * PT],
                                        oh[:, c * PT:(c + 1) * PT], identb[:])
                ohT = lpool.tile([PT, JKP], dt.bfloat16, tag="ohT")
                nc.scalar.copy(ohT[:], ohT_ps[:])
                ohT_t[t] = ohT

            def st_gather(t):
                ohT = ohT_t.pop(t)
                g_ps = psA.tile([PT, 51], dt.float32, tag="gps")
                for c in range(4):
                    for si, s in enumerate((t_hi, t_mid, t_lo)):
                        nc.tensor.matmul(g_ps[:], ohT[:, c * PT:(c + 1) * PT],
                                         s[:, c * 51:(c + 1) * 51],
                                         start=(c == 0 and si == 0),
                                         stop=(c == 3 and si == 2))
                slot = t % 4
                if slot == 0:
                    gst_ref[0] = lpool.tile([PT, 4 * 51], dt.float32, tag="gst",
                                            name="gst")
                gst = gst_ref[0]
                nc.scalar.copy(gst[:, slot * 51:(slot + 1) * 51], g_ps[:])
                if slot == 3 or t == ntiles - 1:
                    nb = slot + 1
                    t0 = t - slot
                    nc.sync.dma_start(
                        bass.AP(out_d[:].tensor, t0 * PT * 51,
                                [[51, PT], [PT * 51, nb], [1, 51]]),
                        bass.AP(gst.tensor, 0, [[4 * 51, PT], [51, nb], [1, 51]]))

            for t in range(ntiles + 4):
                if t < ntiles:
                    st_score(t)
                if 1 <= t < ntiles + 1:
                    st_rmin(t - 1)
                if 2 <= t < ntiles + 2:
                    st_islt(t - 2)
                if 3 <= t < ntiles + 3:
                    st_transp(t - 3)
                if t >= 4:
                    st_gather(t - 4)

    nc.compile()
    return nc


# --------------------------------------------------------------------------
# host-side constants / shards
# --------------------------------------------------------------------------
def _build_consts():
    import ml_dtypes
    c = {}
    c["identf"] = np.eye(PT, dtype=np.float32)
    c["identb"] = np.eye(PT, dtype=np.float32).astype(ml_dtypes.bfloat16)
    s = (np.arange(SW)[:, None] * RPC + np.arange(RPC)[None, :]).reshape(-1)
    c["revconst"] = np.broadcast_to((2047 - s).astype(np.float32),
                                    (PT, SLABF)).copy()
    mtx = np.zeros((PT, 4, 51), np.float32)
    mty = np.zeros((PT, 4, 51), np.float32)
    mtc = np.zeros((PT, 4, 51), np.float32)
    for ch in range(4):
        for p in range(PT):
            jk = ch * PT + p
            if jk < JK:
                j = jk // K
                mtx[p, ch, j] = 1.0
                mty[p, ch, 17 + j] = 1.0
                mtc[p, ch, 34 + j] = 1.0
    c["maskTx"] = mtx.reshape(PT, 4 * 51).copy()
    c["maskTy"] = mty.reshape(PT, 4 * 51).copy()
    c["maskTc"] = mtc.reshape(PT, 4 * 51).copy()
    c["wzero"] = np.zeros((CAUG, JKP), np.float32)
    c["czero"] = np.zeros((2, JKP), np.float32)
    c["cgidx"] = np.broadcast_to(
        np.repeat(np.arange(NSLAB, dtype=np.float32), 8), (J, NSLAB * 8)).copy()
    return c


def _prep_shards(poses, heat, off):
    consts = _build_consts()
    heat_pad = np.full((J, H + 4, W + 4), -1.0, np.float32)
    heat_pad[:, 2:-2, 2:-2] = heat
    in_maps = []
    for core in range(NCORES):
        r0 = core * RPC
        lo = core * NPAD
        ps = poses[min(lo, len(poses)):min(lo + NPAD, len(poses))]
        pa = np.zeros((NPAD, CAUG), np.float32)
        if len(ps):
            pa[:len(ps), 0:17] = ps[:, 0::2]
            pa[:len(ps), 17:34] = ps[:, 1::2]
        pa[:, 34] = 1.0
        slab = np.full((NTILE_H * PT, SLABW, RW), -1.0, np.float32)
        for cg in range(NSLAB):
            tile_i, cg_l = divmod(cg, SPT)
            c0 = cg * SW
            ncol = min(SLABW, W + 4 - c0)
            blk = heat_pad[:, r0:r0 + RW, c0:c0 + ncol]       # [J, 68, ncol]
            for j in range(J):
                p = tile_i * PT + cg_l * J + j
                slab[p, :ncol, :] = blk[j].T
        m = {
            "poses": pa,
            "heat": slab.reshape(NTILE_H * PT, SLABW * RW),
            "offs": np.ascontiguousarray(
                off[:, :, r0:r0 + RPC, :].transpose(2, 3, 0, 1)
            ).reshape(OFFSZ, 1),
            "coreconst": np.broadcast_to(
                np.array([r0 * W, r0], np.float32), (J, 2)).copy(),
        }
        m.update(consts)
        in_maps.append(m)
    return in_maps


def _fixup(out_full, cnt, cand, poses):
    """Recompute sites where the one-hot matched != 1 candidate, exactly."""
    hx = cand[:, 0:K]
    hy = cand[:, K:2 * K]
    bad = np.argwhere(np.abs(cnt - 1.0) > 0.25)
    for n, j in bad:
        if n >= len(poses):
            continue
        px = np.float32(poses[n, 2 * j])
        py = np.float32(poses[n, 2 * j + 1])
        dx = (px - hx[j]).astype(np.float32)
        dy = (py - hy[j]).astype(np.float32)
        d2 = (dx * dx + dy * dy).astype(np.float32)
        kk = int(np.argmin(d2))
        out_full[n, 2 * j] = hx[j, kk]
        out_full[n, 2 * j + 1] = hy[j, kk]
    return out_full


def kernel(poses, heat_pred, offset_pred, stride):
    from concourse.bass_utils import run_bass_kernel_spmd

    poses = np.asarray(poses, dtype=np.float32)
    heat_pred = np.asarray(heat_pred, dtype=np.float32)
    offset_pred = np.asarray(offset_pred, dtype=np.float32)
    stride_v = int(np.asarray(stride).reshape(-1)[0]) if np.ndim(stride) else int(stride)

    key = ("prog", stride_v)
    if key not in _CACHE:
        _CACHE[key] = _build_program(stride_v)
    nc = _CACHE[key]

    in_maps = _prep_shards(poses, heat_pred, offset_pred)
    r = run_bass_kernel_spmd(nc, in_maps, list(range(NCORES)))
    global LAST_EXEC_NS
    LAST_EXEC_NS = r.exec_time_ns
    res = r.results

    outs = []
    cand = np.asarray(res[0]["cand"], dtype=np.float32)
    for core in range(NCORES):
        o = np.asarray(res[core]["out"], dtype=np.float32)   # [NPAD, 51]
        outs.append(o)
    N = len(poses)
    full = np.zeros((N, 2 * J), np.float32)
    cnt_full = np.zeros((N, J), np.float32)
    for core in range(NCORES):
        lo = core * NPAD
        hi = min(lo + NPAD, N)
        if hi <= lo:
            break
        o = outs[core][:hi - lo]
        full[lo:hi, 0::2] = o[:, 0:17]
        full[lo:hi, 1::2] = o[:, 17:34]
        cnt_full[lo:hi] = o[:, 34:51]
    full = _fixup(full, cnt_full, cand, poses)
    return full
